# revision 22
# baseline (speedup 1.0000x reference)
# Trainium2 Bass kernel for nn_MicroVideoRec (segment_reduce).
#
# Strategy (8 NeuronCores, SPMD):
#   Host: argsort interactions by item_id (the sharding permutation), shard
#     by item-id range: core k owns bins [125056*k, 125056*(k+1)).  Each
#     core's element stream is cut into 128 partition rows at bin
#     boundaries (977 bins/row) and padded to a fixed width with sentinel
#     ids; ids are stored row-relative (0..976) so each partition handles
#     its own 977-bin range independently.
#   Device (per core): stream (brel, signal, rep) rows tile by tile;
#     segmented scans (tensor_tensor_scan with reset masks) produce, at
#     the last element of every run: run length, run sums of signal/rep,
#     and run max/min of signal (offset positive).  A gpsimd local_scatter
#     per field writes those values (fp32 as two u16 halves) to the run's
#     bin slot inside a per-tile SBUF tile; non-boundary elements carry
#     index -1 and are skipped.  Per-tile results are summed into resident
#     fp32 accumulators (each bin is written by exactly one tile).  Dense
#     epilogue computes both outputs; a tiny AllReduce collective shares
#     the rep_log sum/sumsq for the global mean/std.
#   Host: concatenates the 8 per-core [2, 125056] outputs, trims to 1M.
import sys
import numpy as np

try:
    import concourse.bass as bass
except ImportError:  # pragma: no cover
    sys.path.insert(0, "/opt/trn_rl_repo")
    import concourse.bass as bass

import concourse.bacc as bacc
import concourse.tile as tile
from concourse import library_config, mybir
from concourse.bass_utils import run_bass_kernel_spmd

P = 128                 # SBUF partitions
NCORES = 8
NUM_ITEMS = 1_000_000
BINS_PER_ROW = 977      # bins covered by one partition row
CORE_BINS = P * BINS_PER_ROW          # 125056 bins per core
TOTAL_BINS = NCORES * CORE_BINS       # 1000448 >= NUM_ITEMS
W = 1024                # elements per partition per tile
NT = 20                 # tiles
F = W * NT              # row capacity (20480)
NFIELD = 5              # cnt, sig_sum, rep_sum, maxp, minp
NELEM = 2 * (BINS_PER_ROW + 1)   # u16 slots per partition in scatter dst
SENT_LO = -1            # leading sentinel (row start)
SENT_HI = 1800          # trailing sentinel (pad), > BINS_PER_ROW
OFFS = 16.0             # shift making signal max/min scans positive

f32 = mybir.dt.float32
i32 = mybir.dt.int32
i16 = mybir.dt.int16
ALU = mybir.AluOpType
ACT = mybir.ActivationFunctionType


def build_nc(repeat=1):
    nc = bacc.Bacc("TRN2", target_bir_lowering=False, debug=False,
                   num_devices=NCORES)

    ids_in = nc.dram_tensor("ids_in", [P, F + 2], i16, kind="ExternalInput").ap()
    sig_in = nc.dram_tensor("sig_in", [P, F], f32, kind="ExternalInput").ap()
    rep_in = nc.dram_tensor("rep_in", [P, F], f32, kind="ExternalInput").ap()
    lam_in = nc.dram_tensor("lam_in", [P, 1], f32, kind="ExternalInput").ap()

    cc_in = nc.dram_tensor("cc_in", [1, 16], f32).ap()
    cc_out = nc.dram_tensor("cc_out", [1, 16], f32, addr_space="Shared").ap()
    out_d = nc.dram_tensor("out_d", [2, CORE_BINS], f32,
                           kind="ExternalOutput").ap()

    with tile.TileContext(nc) as tc:
        with tc.tile_pool(name="const", bufs=1) as const_p, \
             tc.tile_pool(name="small", bufs=1) as small_p:
            nc.gpsimd.load_library(library_config.local_scatter)

            neg1_t = const_p.tile([P, W], i16)
            nc.vector.memset(neg1_t[:], -1)
            one16_t = const_p.tile([P, W], i16)
            nc.vector.memset(one16_t[:], 1)
            ones_t = const_p.tile([P, W], f32)
            nc.vector.memset(ones_t[:], 1.0)
            one_bias_t = const_p.tile([P, 1], f32)
            nc.vector.memset(one_bias_t[:], 1.0)
            ones_col = const_p.tile([P, 1], f32)
            nc.vector.memset(ones_col[:], 1.0)
            ones_row = const_p.tile([1, P], f32)
            nc.vector.memset(ones_row[:], 1.0)

            lamraw_t = small_p.tile([P, 1], f32)
            nc.sync.dma_start(lamraw_t[:], lam_in)
            lam_t = small_p.tile([P, 1], f32)
            nc.scalar.activation(lam_t[:], lamraw_t[:], ACT.Sigmoid)

            for _rep in range(repeat):
                _build_body(nc, tc, ids_in, sig_in, rep_in, cc_in, cc_out,
                            out_d, neg1_t, one16_t, ones_t, one_bias_t,
                            ones_col, ones_row, lam_t)
    nc.compile()
    return nc


def _build_body(nc, tc, ids_in, sig_in, rep_in, cc_in, cc_out, out_d,
                neg1_t, one16_t, ones_t, one_bias_t, ones_col, ones_row,
                lam_t, dbg_d=None):
    with tc.tile_pool(name="acc", bufs=1) as acc_p:
        acc = []
        for fj in range(NFIELD):
            a = acc_p.tile([P, NELEM // 2], f32, name=f"acc{fj}")
            nc.vector.memset(a[:], 0.0)
            acc.append(a)

        with tc.tile_pool(name="in", bufs=3) as in_p, \
             tc.tile_pool(name="work", bufs=2) as work_p, \
             tc.tile_pool(name="scan", bufs=2) as scan_p, \
             tc.tile_pool(name="dst", bufs=2) as dst_p:
            prev_scans = None
            for t in range(NT):
                ids_t = in_p.tile([P, W + 2], i16, tag="ids")
                nc.sync.dma_start(ids_t[:], ids_in[:, t * W: t * W + W + 2])
                sig_t = in_p.tile([P, W], f32, tag="sig")
                nc.sync.dma_start(sig_t[:], sig_in[:, t * W: (t + 1) * W])
                rep_t = in_p.tile([P, W], f32, tag="rep")
                nc.sync.dma_start(rep_t[:], rep_in[:, t * W: (t + 1) * W])

                eq_t = work_p.tile([P, W], f32, tag="eq")
                nc.vector.tensor_tensor(
                    out=eq_t[:], in0=ids_t[:, 0:W], in1=ids_t[:, 1:W + 1],
                    op=ALU.is_equal)
                meq = eq_t[:]
                lasti_t = work_p.tile([P, W], i16, tag="lasti")
                nc.vector.tensor_tensor(
                    out=lasti_t[:], in0=ids_t[:, 1:W + 1],
                    in1=ids_t[:, 2:W + 2], op=ALU.not_equal)

                # scans: state = (meq * state) op1 data1
                scans = [scan_p.tile([P, W], f32, tag=f"sc{j}",
                                     name=f"sc{j}_{t}")
                         for j in range(NFIELD)]

                def carry(j, _prev=prev_scans):
                    if _prev is None:
                        return 0.0
                    return _prev[j][:, W - 1:W]

                nc.vector.tensor_tensor_scan(
                    out=scans[0][:], data0=meq, data1=ones_t[:],
                    initial=carry(0), op0=ALU.mult, op1=ALU.add)
                nc.vector.tensor_tensor_scan(
                    out=scans[1][:], data0=meq, data1=sig_t[:],
                    initial=carry(1), op0=ALU.mult, op1=ALU.add)
                nc.vector.tensor_tensor_scan(
                    out=scans[2][:], data0=meq, data1=rep_t[:],
                    initial=carry(2), op0=ALU.mult, op1=ALU.add)
                xp_t = work_p.tile([P, W], f32, tag="xp")
                nc.vector.tensor_scalar(
                    out=xp_t[:], in0=sig_t[:], scalar1=OFFS, scalar2=None,
                    op0=ALU.add)
                nc.vector.tensor_tensor_scan(
                    out=scans[3][:], data0=meq, data1=xp_t[:],
                    initial=carry(3), op0=ALU.mult, op1=ALU.max)
                xm_t = work_p.tile([P, W], f32, tag="xm")
                nc.vector.tensor_scalar(
                    out=xm_t[:], in0=sig_t[:], scalar1=-1.0, scalar2=OFFS,
                    op0=ALU.mult, op1=ALU.add)
                nc.vector.tensor_tensor_scan(
                    out=scans[4][:], data0=meq, data1=xm_t[:],
                    initial=carry(4), op0=ALU.mult, op1=ALU.max)

                # index pairs (2b, 2b+1) for the u16-halves scatter; -1 rows
                # (non-boundary / pad elements) become (-2, -1): skipped.
                brel_t = work_p.tile([P, W], i16, tag="brel")
                nc.vector.tensor_copy(out=brel_t[:], in_=neg1_t[:])
                nc.vector.copy_predicated(out=brel_t[:], mask=lasti_t[:],
                                          data=ids_t[:, 1:W + 1])
                idx2_t = work_p.tile([P, 2 * W], i16, tag="idx2")
                iv = idx2_t[:].rearrange("p (w two) -> p w two", two=2)
                b2 = iv[:, :, 0]
                nc.vector.tensor_tensor(out=b2, in0=brel_t[:], in1=brel_t[:],
                                        op=ALU.add)
                nc.vector.tensor_tensor(out=iv[:, :, 1], in0=b2,
                                        in1=one16_t[:], op=ALU.add)

                dsts = [dst_p.tile([P, NELEM], i16, tag=f"d{j}",
                                   name=f"d{j}_{t}")
                        for j in range(NFIELD)]
                for j in range(NFIELD):
                    nc.gpsimd.local_scatter(
                        out_ap=dsts[j][:],
                        data_ap=scans[j][:].bitcast(i16),
                        idxs_ap=idx2_t[:],
                        channels=P, num_elems=NELEM, num_idxs=2 * W)
                    nc.vector.tensor_tensor(
                        out=acc[j][:], in0=acc[j][:],
                        in1=dsts[j][:].bitcast(f32), op=ALU.add)
                prev_scans = scans

        # ---- epilogue ----
        with tc.tile_pool(name="epi", bufs=1) as epi_p, \
             tc.tile_pool(name="psum", bufs=1, space="PSUM") as psum_p:
            B = BINS_PER_ROW
            cnt = acc[0][:, 0:B]
            ssig = acc[1][:, 0:B]
            srep = acc[2][:, 0:B]
            mxp = acc[3][:, 0:B]
            mnp = acc[4][:, 0:B]

            safe_t = epi_p.tile([P, B], f32)
            nc.vector.tensor_scalar(out=safe_t[:], in0=cnt, scalar1=1.0,
                                    scalar2=None, op0=ALU.max)
            inv_t = epi_p.tile([P, B], f32)
            nc.vector.reciprocal(inv_t[:], safe_t[:])
            sigmean_t = epi_p.tile([P, B], f32)
            nc.vector.tensor_tensor(out=sigmean_t[:], in0=ssig, in1=inv_t[:],
                                    op=ALU.mult)
            repmean_t = epi_p.tile([P, B], f32)
            nc.vector.tensor_tensor(out=repmean_t[:], in0=srep, in1=inv_t[:],
                                    op=ALU.mult)
            M_t = epi_p.tile([P, B], f32)
            nc.vector.tensor_scalar(out=M_t[:], in0=mxp, scalar1=-OFFS,
                                    scalar2=None, op0=ALU.add)
            m_t = epi_p.tile([P, B], f32)
            nc.vector.tensor_scalar(out=m_t[:], in0=mnp, scalar1=-1.0,
                                    scalar2=OFFS, op0=ALU.mult, op1=ALU.add)
            absM_t = epi_p.tile([P, B], f32)
            nc.scalar.activation(absM_t[:], M_t[:], ACT.Abs)
            absm_t = epi_p.tile([P, B], f32)
            nc.scalar.activation(absm_t[:], m_t[:], ACT.Abs)
            ge_t = epi_p.tile([P, B], i32)
            nc.vector.tensor_tensor(out=ge_t[:], in0=absM_t[:],
                                    in1=absm_t[:], op=ALU.is_ge)
            maxabs_t = epi_p.tile([P, B], f32)
            nc.vector.tensor_copy(out=maxabs_t[:], in_=m_t[:])
            nc.vector.copy_predicated(out=maxabs_t[:], mask=ge_t[:],
                                      data=M_t[:])
            nonempty_t = epi_p.tile([P, B], f32)
            nc.vector.tensor_scalar(out=nonempty_t[:], in0=cnt, scalar1=0.0,
                                    scalar2=None, op0=ALU.is_gt)
            nc.vector.tensor_tensor(out=maxabs_t[:], in0=maxabs_t[:],
                                    in1=nonempty_t[:], op=ALU.mult)
            sigfull_t = epi_p.tile([P, B], f32)
            nc.vector.scalar_tensor_tensor(
                out=sigfull_t[:], in0=maxabs_t[:], scalar=lam_t[:],
                in1=sigmean_t[:], op0=ALU.mult, op1=ALU.add)
            nc.sync.dma_start(out_d[0].rearrange("(p j) -> p j", p=P),
                              sigfull_t[:])

            replog_t = epi_p.tile([P, B], f32)
            nc.scalar.activation(replog_t[:], repmean_t[:], ACT.Ln,
                                 bias=one_bias_t[:])
            s1_t = epi_p.tile([P, 1], f32)
            nc.vector.tensor_reduce(out=s1_t[:], in_=replog_t[:],
                                    axis=mybir.AxisListType.X, op=ALU.add)
            sq_t = epi_p.tile([P, B], f32)
            nc.vector.tensor_tensor(out=sq_t[:], in0=replog_t[:],
                                    in1=replog_t[:], op=ALU.mult)
            s2_t = epi_p.tile([P, 1], f32)
            nc.vector.tensor_reduce(out=s2_t[:], in_=sq_t[:],
                                    axis=mybir.AxisListType.X, op=ALU.add)
            s12_t = epi_p.tile([P, 16], f32)
            nc.vector.memset(s12_t[:], 0.0)
            nc.vector.tensor_copy(out=s12_t[:, 0:1], in_=s1_t[:])
            nc.vector.tensor_copy(out=s12_t[:, 1:2], in_=s2_t[:])
            red_ps = psum_p.tile([1, 16], f32, space="PSUM")
            nc.tensor.matmul(out=red_ps[:], lhsT=ones_col[:], rhs=s12_t[:],
                             start=True, stop=True)
            red_sb = epi_p.tile([1, 16], f32)
            nc.vector.tensor_copy(out=red_sb[:], in_=red_ps[:])
            nc.sync.dma_start(cc_in, red_sb[:])
            nc.gpsimd.collective_compute(
                "AllReduce", ALU.add,
                replica_groups=[list(range(NCORES))],
                ins=[cc_in], outs=[cc_out])
            tot_sb = epi_p.tile([1, 16], f32)
            nc.sync.dma_start(tot_sb[:], cc_out)
            tot_ps = psum_p.tile([P, 16], f32, space="PSUM")
            nc.tensor.matmul(out=tot_ps[:], lhsT=ones_row[:], rhs=tot_sb[:],
                             start=True, stop=True)
            tot_t = epi_p.tile([P, 16], f32)
            nc.vector.tensor_copy(out=tot_t[:], in_=tot_ps[:])

            NB = float(NUM_ITEMS)
            mean_t = epi_p.tile([P, 1], f32)
            nc.vector.tensor_scalar(out=mean_t[:], in0=tot_t[:, 0:1],
                                    scalar1=1.0 / NB, scalar2=None,
                                    op0=ALU.mult)
            m2s_t = epi_p.tile([P, 1], f32)
            nc.vector.tensor_tensor(out=m2s_t[:], in0=mean_t[:],
                                    in1=tot_t[:, 0:1], op=ALU.mult)
            var_t = epi_p.tile([P, 1], f32)
            nc.vector.tensor_tensor(out=var_t[:], in0=tot_t[:, 1:2],
                                    in1=m2s_t[:], op=ALU.subtract)
            nc.vector.tensor_scalar(out=var_t[:], in0=var_t[:],
                                    scalar1=1.0 / (NB - 1.0), scalar2=None,
                                    op0=ALU.mult)
            std_t = epi_p.tile([P, 1], f32)
            nc.scalar.sqrt(std_t[:], var_t[:])
            nc.vector.tensor_scalar(out=std_t[:], in0=std_t[:], scalar1=1e-6,
                                    scalar2=None, op0=ALU.add)
            istd_t = epi_p.tile([P, 1], f32)
            nc.vector.reciprocal(istd_t[:], std_t[:])
            repsc_t = epi_p.tile([P, B], f32)
            nc.vector.tensor_scalar(out=repsc_t[:], in0=replog_t[:],
                                    scalar1=mean_t[:], scalar2=None,
                                    op0=ALU.subtract)
            nc.vector.tensor_scalar(out=repsc_t[:], in0=repsc_t[:],
                                    scalar1=istd_t[:], scalar2=None,
                                    op0=ALU.mult)
            nc.sync.dma_start(out_d[1].rearrange("(p j) -> p j", p=P),
                              repsc_t[:])
            if dbg_d is not None:
                for fj in range(NFIELD):
                    nc.sync.dma_start(
                        dbg_d[fj].rearrange("(p j) -> p j", p=P),
                        acc[fj][:, 0:B])


def host_prep(item_ids, signals, reps):
    """Sort by id, shard by bin range, cut rows at bin boundaries, pad.

    Row-relative ids: brel = id - BINS_PER_ROW * global_row in [0, 977).
    """
    ids = np.ascontiguousarray(np.asarray(item_ids).astype(np.int32))
    sig = np.ascontiguousarray(np.asarray(signals, dtype=np.float32))
    rep = np.ascontiguousarray(np.asarray(reps, dtype=np.float32))

    order = np.argsort(ids)
    ids_s = ids[order]
    sig_s = sig[order]
    rep_s = rep[order]

    nrows = NCORES * P
    cuts = np.searchsorted(
        ids_s, np.arange(nrows + 1, dtype=np.int64) * BINS_PER_ROW)
    row_len = np.diff(cuts)
    assert row_len.max() <= F, f"row overflow: {row_len.max()} > {F}"

    ids_arr = np.full((nrows, F + 2), SENT_HI, np.int16)
    ids_arr[:, 0] = SENT_LO
    sig_arr = np.zeros((nrows, F), np.float32)
    rep_arr = np.zeros((nrows, F), np.float32)
    for r in range(nrows):
        lo, hi = cuts[r], cuts[r + 1]
        n = hi - lo
        if n:
            ids_arr[r, 1:1 + n] = (ids_s[lo:hi] - r * BINS_PER_ROW).astype(np.int16)
            sig_arr[r, :n] = sig_s[lo:hi]
            rep_arr[r, :n] = rep_s[lo:hi]
    return ids_arr, sig_arr, rep_arr


_NC_CACHE = {}


def _get_nc(repeat=1):
    if repeat not in _NC_CACHE:
        _NC_CACHE[repeat] = build_nc(repeat)
    return _NC_CACHE[repeat]


def make_in_maps(item_ids, signals, reps, lam_raw):
    ids_arr, sig_arr, rep_arr = host_prep(item_ids, signals, reps)
    lam_vec = np.full((P, 1), float(np.asarray(lam_raw)), np.float32)
    in_maps = []
    for k in range(NCORES):
        rs = slice(k * P, (k + 1) * P)
        in_maps.append({
            "ids_in": np.ascontiguousarray(ids_arr[rs]),
            "sig_in": np.ascontiguousarray(sig_arr[rs]),
            "rep_in": np.ascontiguousarray(rep_arr[rs]),
            "lam_in": lam_vec,
        })
    return in_maps


def run_maps(in_maps, repeat=1):
    nc = _get_nc(repeat)
    res = run_bass_kernel_spmd(nc, in_maps, core_ids=list(range(NCORES)),
                               trace=False)
    outs = [res.results[k]["out_d"] for k in range(NCORES)]
    return np.concatenate(outs, axis=1)[:, :NUM_ITEMS].astype(np.float32)


def kernel(item_ids, signals, reps, lam_raw, num_items=None, _repeat=1):
    if num_items is not None:
        assert int(num_items) == NUM_ITEMS
    return run_maps(make_in_maps(item_ids, signals, reps, lam_raw), _repeat)
